# revision 12
# baseline (speedup 1.0000x reference)
"""Trainium2 Bass kernel for nn_MixedLoss (prototype + pairwise + contrastive loss).

Design: fully replicated, rotation-sharded, collective-free.

Each core receives the FULL embedding tensor, but with rows re-ordered so
that core k's own 512 contrastive rows (256 support + 256 query of classes
[16k, 16k+16)) appear FIRST.  Under this rotation the SPMD program is
identical on every core (no runtime branches, no collectives): the "own"
class windows sit at fixed column offsets, prototypes/classes appear in a
per-core rotated order (softmax/pairwise losses are permutation-invariant),
and the per-core scalar partials are combined on the host.

Engine plan per core:
  L) em[0] -> X via Sync HWDGE, em[1] -> Y via Scalar HWDGE (parallel
     queues), X += Y on DVE per group; squares/row-norms on ACT (Square with
     accum); x_hat = X*rn bf16 on DVE; bounce through DRAM and
     dma_start_transpose into xT layout [128 d-half, 4096 rows] bf16.
  P) fp32 PE transposes of the 4 strip blocks (dist path stays fp32),
     prototypes via tiny PE matmuls with a group-selector, pairwise loss,
     prototype dists/prob (fp32).
  Z) z = x_hatT.T @ x_hatT in bf16 (PSUM pieces [128, 2048]); e = exp(z-10)
     on ACT with row-sum accum; e^2 row sums on GpSimd (stt accum over the
     bf16 e strip); class-group sums of e on DVE; own-class windows at fixed
     offsets with constant masks.  All sums keep the diagonal; the algebra
     subtracts e_ii once at the end (A = own' - e_ii, S = tot - own', ...).
  F) Batched [128, 4] column math, Ln, stats stack, column-sum matmul,
     DMA out 16 floats; host combines 8x16 stats.
"""

import sys

sys.path.insert(0, "/opt/trn_rl_repo")

import numpy as np

import concourse.bass as bass
import concourse.bacc as bacc
import concourse.tile as tile
from concourse import mybir
from concourse.bass_utils import run_bass_kernel_spmd

F32 = mybir.dt.float32
BF16 = mybir.dt.bfloat16
AF = mybir.ActivationFunctionType
OP = mybir.AluOpType
AX = mybir.AxisListType

NCORES = 8
NWAY, KSHOT, QSHOT, REPEAT, DIM = 128, 16, 16, 2, 256
ROWS = 512            # rows per core strip (256 support + 256 query)
BSZ = 4096
NBLK = 32             # X layout blocks of 128 rows
ALPHA, TEMP, BETA, GAMMA = 0.5, 0.1, 0.1, 0.1
INV_T = 1.0 / TEMP    # 10.0

NSTAT = 12


def _emit(nc):
    em = nc.declare_dram_parameter("em", [2, BSZ, DIM], F32, isOutput=False)
    # fp32 consts: 0 I128, 1 OMI, 2-3 BD|BD, 4 OH(par0 cols0:32, par1 32:64),
    # 5 OWNP par0, 6 OWNP par1, 7 Sel (cols 0:8)
    cf_d = nc.declare_dram_parameter("cf", [8, 128, 128], F32, isOutput=False)
    # bf16 consts: 0-1 BD|BD, 2 OMI
    cb_d = nc.declare_dram_parameter("cb", [3, 128, 128], BF16, isOutput=False)
    stats_d = nc.declare_dram_parameter("stats", [16], F32, isOutput=True)
    dbg_d = nc.declare_dram_parameter("dbg", [128, 16], F32, isOutput=True)

    with tile.TileContext(nc) as tc:
        with (
            tc.tile_pool(name="singles", bufs=1) as singles,
            tc.tile_pool(name="work", bufs=3) as work,
            tc.tile_pool(name="small", bufs=3) as small,
            tc.tile_pool(name="keep", bufs=1) as keep,
            tc.tile_pool(name="epool", bufs=2) as epool,
            tc.tile_pool(name="dump", bufs=2) as dump,
            tc.tile_pool(name="dram", bufs=1, space="DRAM") as dram,
        ):
            # ------------- persistent tiles -------------
            X = singles.tile([128, NBLK * DIM], F32, tag="X")          # 4 MB
            Y = singles.tile([128, NBLK * DIM], F32, tag="Y")          # 4 MB
            xh = singles.tile([128, NBLK * DIM], BF16, tag="xh")       # 2 MB
            xhT = [singles.tile([128, BSZ], BF16, tag=f"xhT{h}", name=f"xhT{h}")
                   for h in range(2)]
            xT32 = [singles.tile([128, ROWS], F32, tag=f"xT32{h}", name=f"xT32{h}")
                    for h in range(2)]
            protoT = [keep.tile([128, NWAY], F32, tag=f"protoT{h}",
                                name=f"protoT{h}") for h in range(2)]
            nsq = keep.tile([128, NBLK], F32, tag="nsq")
            rn = keep.tile([128, NBLK], F32, tag="rn")
            xh_dram = dram.tile([BSZ, DIM], BF16, tag="xh_dram")

            # ============ L: loads (sync HWDGE + gpsimd SWDGE queues) ============
            for g in range(4):
                ev = em[:, 1024 * g:1024 * g + 1024, :].rearrange(
                    "r (j p) d -> r p j d", p=128)
                nc.sync.dma_start(
                    out=X[:, 2048 * g:2048 * g + 2048].rearrange(
                        "p (j d) -> p j d", d=DIM), in_=ev[0])
                nc.gpsimd.dma_start(
                    out=Y[:, 2048 * g:2048 * g + 2048].rearrange(
                        "p (j d) -> p j d", d=DIM), in_=ev[1])

            # ------------- constants (batched, scalar queue) -------------
            CF = singles.tile([128, 8 * 128], F32, tag="CF")
            CB = singles.tile([128, 3 * 128], BF16, tag="CB")
            nc.sync.dma_start(
                out=CF.rearrange("p (s c) -> p s c", c=128),
                in_=cf_d.rearrange("s p c -> p s c"))
            nc.sync.dma_start(
                out=CB.rearrange("p (s c) -> p s c", c=128),
                in_=cb_d.rearrange("s p c -> p s c"))
            I128 = CF[:, 0:128]
            OMI = CF[:, 128:256]
            BDDf = CF[:, 256:512]
            OH = CF[:, 512:576]
            OWNP = [CF[:, 640:768], CF[:, 768:896]]
            SelT = CF[:, 896:904]
            BDDb = CB[:, 0:256]
            OMIb = CB[:, 256:384]
            ones_c = singles.tile([128, 1], F32, tag="ones_c")
            ones_r = singles.tile([1, 128], F32, tag="ones_r")
            nc.vector.memset(ones_c, 1.0)
            nc.vector.memset(ones_r, 1.0)
            stack = singles.tile([128, NSTAT], F32, tag="stack")
            nc.vector.memset(stack, 0.0)
            neg10 = singles.tile([128, 1], F32, tag="neg10")
            nc.vector.memset(neg10, -INV_T)
            neg20 = singles.tile([128, 1], F32, tag="neg20")
            nc.vector.memset(neg20, -2.0 * INV_T)

            # ============ N: repeat-sum, norms, x_hat, transposes ============
            for g in range(4):
                sl = slice(2048 * g, 2048 * g + 2048)
                # alternate the repeat-add between DVE and GpSimd
                eng = nc.vector if g % 2 == 0 else nc.gpsimd
                eng.tensor_add(X[:, sl], X[:, sl], Y[:, sl])
                for j in range(8):
                    b = 8 * g + j
                    sq = dump.tile([128, DIM], F32, tag="sqd")
                    nc.vector.scalar_tensor_tensor(
                        out=sq, in0=X[:, DIM * b:DIM * b + DIM], scalar=1.0,
                        in1=X[:, DIM * b:DIM * b + DIM],
                        op0=OP.mult, op1=OP.mult,
                        accum_out=nsq[:, b:b + 1])
                gs = slice(8 * g, 8 * g + 8)
                rw = small.tile([128, 8], F32, tag="rw")
                nc.scalar.activation(rw, nsq[:, gs], AF.Sqrt, scale=TEMP)
                nc.vector.reciprocal(rn[:, gs], rw)   # rn = sqrt(10)/|X|
                for j in range(8):
                    b = 8 * g + j
                    # x_hat = X * rn (bf16) on ACT (Copy with per-row scale)
                    nc.scalar.mul(xh[:, DIM * b:DIM * b + DIM],
                                  X[:, DIM * b:DIM * b + DIM],
                                  rn[:, b:b + 1])
                nc.gpsimd.dma_start(
                    out=xh_dram[1024 * g:1024 * g + 1024, :].rearrange(
                        "(j p) d -> p j d", p=128),
                    in_=xh[:, 2048 * g:2048 * g + 2048].rearrange(
                        "p (j d) -> p j d", d=DIM))
                for h in range(2):
                    nc.sync.dma_start_transpose(
                        out=xhT[h][:, 1024 * g:1024 * g + 1024],
                        in_=xh_dram[1024 * g:1024 * g + 1024,
                                    128 * h:128 * h + 128])
            qn4all = keep.tile([128, NBLK], F32, tag="qn4all")
            nc.vector.tensor_scalar_mul(qn4all, nsq, 0.25)

            # ================= P: strip transposes, protos =================
            store = {}
            with (
                tc.tile_pool(name="psP", bufs=1, space="PSUM") as psP,
                tc.tile_pool(name="psT", bufs=3, space="PSUM") as psT,
            ):
                # fp32 transposes of the 4 strip blocks (b = 0..3)
                for b in range(4):
                    for h in range(2):
                        pt = psT.tile([128, 128], F32, tag="pst", name="ptrans")
                        nc.tensor.transpose(
                            pt, X[:, DIM * b + 128 * h:DIM * b + 128 * h + 128],
                            I128)
                        nc.scalar.mul(xT32[h][:, 128 * b:128 * b + 128], pt, 1.0)

                # prototypes: for each support block, sum groups of 16 rows
                protoPS = [psP.tile([128, NWAY], F32, tag=f"prps{h}",
                                    name=f"prps{h}") for h in range(2)]
                for g in range(4):
                    for b in range(8 * g, 8 * g + 8):
                        if b % 4 >= 2:
                            continue                 # query block
                        c0 = 16 * (b // 4) + 8 * (b % 4)
                        for h in range(2):
                            nc.tensor.matmul(
                                protoPS[h][:, c0:c0 + 8],
                                lhsT=X[:, DIM * b + 128 * h:
                                       DIM * b + 128 * h + 128],
                                rhs=SelT, start=True, stop=True)
                for h in range(2):
                    # X = 2x and 16 shots -> /32 gives true prototypes
                    nc.vector.tensor_scalar(
                        out=protoT[h], in0=protoPS[h], scalar1=1.0 / 32.0,
                        scalar2=None, op0=OP.mult)

                # ---- pairwise loss (order-invariant) ----
                pn_ps = psT.tile([1, NWAY], F32, tag="pst", name="pn")
                psq = [work.tile([128, NWAY], F32, tag=f"psq{h}", name=f"psq{h}")
                       for h in range(2)]
                for h in range(2):
                    nc.vector.tensor_mul(psq[h], protoT[h], protoT[h])
                for h in range(2):
                    nc.tensor.matmul(pn_ps, lhsT=ones_c, rhs=psq[h],
                                     start=(h == 0), stop=(h == 1))
                pn_row = small.tile([1, NWAY], F32, tag="pn_row")
                nc.vector.tensor_copy(pn_row, pn_ps)
                pnb_ps = psT.tile([128, NWAY], F32, tag="pst", name="pnb_ps")
                nc.tensor.matmul(pnb_ps, lhsT=ones_r, rhs=pn_row,
                                 start=True, stop=True)
                pnb = singles.tile([128, NWAY], F32, tag="pnb")
                nc.vector.tensor_copy(pnb, pnb_ps)

                # ---- prototype dists / prob / loss_pn partials (fp32) ----
                downpB = keep.tile([128, 4], F32, tag="downpB")
                dminB = keep.tile([128, 4], F32, tag="dminB")
                sume_pn = keep.tile([128, 4], F32, tag="sume_pn")
                for b in range(4):
                    par = b % 2
                    dq_ps = psT.tile([128, NWAY], F32, tag="pst", name="dq")
                    for h in range(2):
                        nc.tensor.matmul(
                            dq_ps, lhsT=xT32[h][:, 128 * b:128 * b + 128],
                            rhs=protoT[h], start=(h == 0), stop=(h == 1))
                    # d = nsq/4 + pn - X.p   (x = X/2 -> 2 x.p = X.p)
                    dmat = work.tile([128, NWAY], F32, tag="dmat")
                    nc.vector.tensor_scalar(
                        out=dmat, in0=dq_ps, scalar1=-1.0,
                        scalar2=None, op0=OP.mult)
                    nc.vector.tensor_scalar(
                        out=dmat, in0=dmat, scalar1=1.0,
                        scalar2=qn4all[:, b:b + 1], op0=OP.mult, op1=OP.add)
                    nc.vector.tensor_add(dmat, dmat, pnb)
                    nc.vector.tensor_reduce(dminB[:, b:b + 1], dmat,
                                            axis=AX.X, op=OP.min)
                    probu = work.tile([128, NWAY], F32, tag="probu")
                    nc.scalar.activation(probu, dmat, AF.Exp,
                                         bias=dminB[:, b:b + 1], scale=-1.0,
                                         accum_out=sume_pn[:, b:b + 1])
                    rcp = small.tile([128, 1], F32, tag="rcp")
                    nc.vector.reciprocal(rcp, sume_pn[:, b:b + 1])
                    prob = keep.tile([128, NWAY], F32, tag=f"prob{b}",
                                     name=f"prob{b}")
                    nc.vector.tensor_scalar_mul(prob, probu, rcp)
                    store[f"prob{b}"] = prob
                    scr1 = work.tile([128, 128], F32, tag="scr1")
                    nc.vector.scalar_tensor_tensor(
                        out=scr1, in0=dmat, scalar=1.0, in1=OWNP[par],
                        op0=OP.mult, op1=OP.mult,
                        accum_out=downpB[:, b:b + 1])
                    pown = keep.tile([128, 1], F32, tag=f"pown{b}",
                                     name=f"pown{b}")
                    scr2 = work.tile([128, 128], F32, tag="scr2")
                    nc.vector.scalar_tensor_tensor(
                        out=scr2, in0=prob, scalar=1.0, in1=OWNP[par],
                        op0=OP.mult, op1=OP.mult, accum_out=pown)
                    store[f"pown{b}"] = pown
                    if b >= 2:
                        acc_i = small.tile([128, 1], F32, tag="acc_i")
                        nc.vector.tensor_tensor(
                            out=acc_i, in0=downpB[:, b:b + 1],
                            in1=dminB[:, b:b + 1], op=OP.is_equal)
                        nc.vector.tensor_copy(
                            stack[:, 6 + (b - 2):7 + (b - 2)], acc_i)

            # ===================== Z: contrastive strip =====================
            # Piece-outer sweep: p=0 for all 4 row blocks (cols 0:2048, which
            # contain the own/diag windows), then p=1 (cols 2048:4096).  Lets
            # the Gram start as soon as the first two transpose groups land.
            zdgB = keep.tile([128, 4], F32, tag="zdgB")
            ownzB = keep.tile([128, 4], F32, tag="ownzB")
            owne1B = keep.tile([128, 4], F32, tag="owne1B")
            owne2B = keep.tile([128, 4], F32, tag="owne2B")
            cfullB = keep.tile([128, 4], F32, tag="cfullB")
            sume_all = keep.tile([128, 8], F32, tag="sume_all")
            sume2_all = keep.tile([128, 8], F32, tag="sume2_all")
            zwinB = keep.tile([128, 1024], F32, tag="zwinB")
            estripB = [keep.tile([128, BSZ], BF16, tag=f"estrip{b}",
                                 name=f"estrip{b}") for b in range(4)]
            eg0B = [keep.tile([128, 128], F32, tag=f"eg0_{b}",
                              name=f"eg0_{b}") for b in range(4)]
            with tc.tile_pool(name="psZ", bufs=2, space="PSUM") as psZ:
                for p in range(2):
                    for b in range(4):
                        par = b % 2
                        w0 = 128 * par          # own support window start
                        estrip = estripB[b]
                        zp = psZ.tile([128, 2048], F32, tag="zps", name="zps")
                        for h in range(2):
                            for ch in range(4):
                                nc.tensor.matmul(
                                    zp[:, 512 * ch:512 * ch + 512],
                                    lhsT=xhT[h][:, 128 * b:128 * b + 128],
                                    rhs=xhT[h][:, 2048 * p + 512 * ch:
                                               2048 * p + 512 * ch + 512],
                                    start=(h == 0), stop=(h == 1))
                        nc.scalar.activation(
                            estrip[:, 2048 * p:2048 * p + 2048], zp, AF.Exp,
                            bias=neg10,
                            accum_out=sume_all[:, 2 * b + p:2 * b + p + 1])
                        if p == 0:
                            # e^2 row sums piece 0 on ACT: exp(2z-20)
                            e2d = dump.tile([128, 2048], BF16, tag="e2d")
                            nc.scalar.activation(
                                e2d, zp, AF.Exp, bias=neg20, scale=2.0,
                                accum_out=sume2_all[:, 2 * b:2 * b + 1])
                            # stage own windows (support + query) to SBUF
                            zsrc = zp[:, w0:w0 + 384].rearrange(
                                "p (s c) -> p s c", c=128)[:, 0:3:2, :]
                            zwin = zwinB[:, 256 * b:256 * b + 256]
                            nc.scalar.copy(
                                zwin.rearrange("p (s c) -> p s c", c=128), zsrc)
                            # diag z / own-class z sums
                            dsel = 256 * b + (0 if b < 2 else 128)
                            scr3 = work.tile([128, 128], F32, tag="scr3")
                            nc.vector.scalar_tensor_tensor(
                                out=scr3, in0=zwinB[:, dsel:dsel + 128],
                                scalar=1.0, in1=I128, op0=OP.mult, op1=OP.mult,
                                accum_out=zdgB[:, b:b + 1])
                            scr4 = work.tile([128, 256], F32, tag="scr4")
                            nc.vector.scalar_tensor_tensor(
                                out=scr4, in0=zwin, scalar=1.0, in1=BDDf,
                                op0=OP.mult, op1=OP.mult,
                                accum_out=ownzB[:, b:b + 1])
                            # zero the e diagonal before the group/window sums
                            dwin = 128 * b
                            nc.vector.tensor_mul(estrip[:, dwin:dwin + 128],
                                                 estrip[:, dwin:dwin + 128],
                                                 OMIb)
                            ewin = estrip[:, w0:w0 + 384].rearrange(
                                "p (s c) -> p s c", c=128)[:, 0:3:2, :]
                            scr6 = work.tile([128, 256], BF16, tag="scr6")
                            nc.vector.scalar_tensor_tensor(
                                out=scr6.rearrange("p (s c) -> p s c", c=128),
                                in0=ewin, scalar=1.0, in1=BDDb.rearrange(
                                    "p (s c) -> p s c", c=128),
                                op0=OP.mult, op1=OP.mult,
                                accum_out=owne1B[:, b:b + 1])
                            e2w = work.tile([128, 256], BF16, tag="e2w")
                            nc.vector.tensor_mul(
                                e2w.rearrange("p (s c) -> p s c", c=128),
                                ewin, ewin)
                            scr5 = work.tile([128, 256], BF16, tag="scr5")
                            nc.vector.scalar_tensor_tensor(
                                out=scr5, in0=e2w, scalar=1.0, in1=BDDb,
                                op0=OP.mult, op1=OP.mult,
                                accum_out=owne2B[:, b:b + 1])
                            # class-group sums of e, piece 0 (classes
                            # 0:64): two bf16 tree folds, then a 4-wide reduce
                            ev = estrip[:, 0:2048].rearrange(
                                "p (g s) -> p g s", s=16)
                            tf1 = dump.tile([128, 1024], BF16, tag="tf1")
                            nc.vector.tensor_add(
                                tf1.rearrange("p (g s) -> p g s", s=8),
                                ev[:, :, 0:8], ev[:, :, 8:16])
                            tf1v = tf1.rearrange("p (g s) -> p g s", s=8)
                            tf2 = dump.tile([128, 512], BF16, tag="tf2")
                            nc.vector.tensor_add(
                                tf2.rearrange("p (g s) -> p g s", s=4),
                                tf1v[:, :, 0:4], tf1v[:, :, 4:8])
                            nc.vector.reduce_sum(
                                eg0B[b], tf2.rearrange("p (g s) -> p g s", s=4),
                                axis=AX.X)
                        else:
                            # e^2 row sums piece 1 on DVE: TT mult at 2x,
                            # then single-src ts accumulate at 4x
                            e2d = dump.tile([128, 2048], BF16, tag="e2d")
                            nc.vector.tensor_mul(e2d, estrip[:, 2048:4096],
                                                 estrip[:, 2048:4096])
                            e2s = dump.tile([128, 2048], BF16, tag="e2s")
                            nc.vector.tensor_scalar(
                                out=e2s, in0=e2d, scalar1=1.0, scalar2=0.0,
                                op0=OP.mult, op1=OP.add,
                                accum_out=sume2_all[:, 2 * b + 1:2 * b + 2])
                            ev1 = estrip[:, 2048:4096].rearrange(
                                "p (g s) -> p g s", s=16)
                            tg1 = dump.tile([128, 1024], BF16, tag="tf1")
                            nc.vector.tensor_add(
                                tg1.rearrange("p (g s) -> p g s", s=8),
                                ev1[:, :, 0:8], ev1[:, :, 8:16])
                            tg1v = tg1.rearrange("p (g s) -> p g s", s=8)
                            tg2 = dump.tile([128, 512], BF16, tag="tf2")
                            nc.vector.tensor_add(
                                tg2.rearrange("p (g s) -> p g s", s=4),
                                tg1v[:, :, 0:4], tg1v[:, :, 4:8])
                            eg1 = work.tile([128, 128], F32, tag="eg1")
                            nc.vector.reduce_sum(
                                eg1, tg2.rearrange("p (g s) -> p g s", s=4),
                                axis=AX.X)
                            # fold support/query halves per class
                            egc = work.tile([128, 128], F32, tag="egc")
                            e0v = eg0B[b].rearrange("p (s h c) -> p s h c",
                                                    h=2, c=16)
                            e1v = eg1.rearrange("p (s h c) -> p s h c",
                                                h=2, c=16)
                            nc.vector.tensor_add(
                                egc[:, 0:64].rearrange("p (s c) -> p s c",
                                                       c=16),
                                e0v[:, :, 0, :], e0v[:, :, 1, :])
                            nc.vector.tensor_add(
                                egc[:, 64:128].rearrange("p (s c) -> p s c",
                                                         c=16),
                                e1v[:, :, 0, :], e1v[:, :, 1, :])
                            scr7 = work.tile([128, 128], F32, tag="scr7")
                            nc.vector.scalar_tensor_tensor(
                                out=scr7, in0=egc, scalar=1.0,
                                in1=store[f"prob{b}"],
                                op0=OP.mult, op1=OP.mult,
                                accum_out=cfullB[:, b:b + 1])

            # ===================== F: pairwise tail + final batched math =====================
            # pairwise loss runs after the z loop: its long serial
            # DVE<->PE<->ACT chain must not block the engine FIFOs mid-kernel
            with tc.tile_pool(name="psF", bufs=2, space="PSUM") as psF:
                gp_ps = psF.tile([128, NWAY], F32, tag="psf", name="gp")
                for h in range(2):
                    nc.tensor.matmul(gp_ps, lhsT=protoT[h], rhs=protoT[h],
                                     start=(h == 0), stop=(h == 1))
                gp_sb = work.tile([128, NWAY], F32, tag="gp_sb")
                nc.vector.tensor_copy(gp_sb, gp_ps)
                scrA = work.tile([128, 128], F32, tag="scrA")
                pnd = small.tile([128, 1], F32, tag="pnd")
                nc.vector.scalar_tensor_tensor(
                    out=scrA, in0=gp_sb, scalar=1.0, in1=I128,
                    op0=OP.mult, op1=OP.mult, accum_out=pnd)
                pnd16 = small.tile([128, 1], F32, tag="pnd16")
                nc.vector.tensor_scalar_mul(pnd16, pnd, 1.0 / 16.0)
                # sq = pnd/16 + pn/16 - Gp/8   (= pairwise dist / sqrt(d))
                sqm = work.tile([128, NWAY], F32, tag="sqm")
                nc.vector.tensor_scalar(
                    out=sqm, in0=gp_sb, scalar1=-0.125, scalar2=pnd16,
                    op0=OP.mult, op1=OP.add)
                pnb16 = work.tile([128, NWAY], F32, tag="pnb16")
                nc.vector.tensor_scalar_mul(pnb16, pnb, 1.0 / 16.0)
                nc.vector.tensor_add(sqm, sqm, pnb16)
                nc.vector.tensor_mul(sqm, sqm, OMI)       # zero diagonal
                t1c = small.tile([128, 1], F32, tag="t1c")
                t2c = small.tile([128, 1], F32, tag="t2c")
                nc.vector.reduce_sum(t1c, sqm, axis=AX.X)
                scrB = work.tile([128, 128], F32, tag="scrB")
                nc.vector.scalar_tensor_tensor(
                    out=scrB, in0=sqm, scalar=1.0, in1=sqm,
                    op0=OP.mult, op1=OP.mult, accum_out=t2c)
                t1_ps = psF.tile([1, 1], F32, tag="psf1", name="t1s")
                t2_ps = psF.tile([1, 1], F32, tag="psf1", name="t2s")
                nc.tensor.matmul(t1_ps, lhsT=t1c, rhs=ones_c, start=True, stop=True)
                nc.tensor.matmul(t2_ps, lhsT=t2c, rhs=ones_c, start=True, stop=True)
                NOFF = float(NWAY * NWAY - NWAY)
                t1s = small.tile([1, 1], F32, tag="t1sb")
                nc.vector.tensor_copy(t1s, t1_ps)
                t1sq = small.tile([1, 1], F32, tag="t1sq")
                nc.vector.tensor_mul(t1sq, t1s, t1s)
                var = small.tile([1, 1], F32, tag="var")
                nc.vector.tensor_scalar(out=var, in0=t1sq, scalar1=-1.0 / NOFF,
                                        scalar2=None, op0=OP.mult)
                nc.vector.tensor_add(var, var, t2_ps)
                nc.vector.tensor_scalar_mul(var, var, 1.0 / (NOFF - 1.0))
                sd = small.tile([1, 1], F32, tag="sd")
                nc.scalar.activation(sd, var, AF.Sqrt)
                nc.vector.tensor_scalar_mul(sd, sd, -1.0)
                nrstd = small.tile([1, 1], F32, tag="nrstd")
                nc.vector.reciprocal(nrstd, sd)           # -1/std
                nrb_ps = psF.tile([128, 1], F32, tag="psf1", name="nrb")
                nc.tensor.matmul(nrb_ps, lhsT=ones_r, rhs=nrstd,
                                 start=True, stop=True)
                nrb = small.tile([128, 1], F32, tag="nrb_sb")
                nc.vector.tensor_copy(nrb, nrb_ps)
                # W = exp(-sq/std); row sums (diag gives exp(0)=1, host -128)
                wmat = work.tile([128, NWAY], F32, tag="wmat")
                wsum = keep.tile([128, 1], F32, tag="wsum")
                nc.scalar.activation(wmat, sqm, AF.Exp, scale=nrb,
                                     accum_out=wsum)
                nc.vector.tensor_copy(stack[:, 8:9], wsum)

                e_iiB = small.tile([128, 4], F32, tag="e_iiB")
                nc.scalar.activation(e_iiB, zdgB, AF.Exp, bias=neg10)
                sv = sume_all.rearrange("p (b t) -> p b t", t=2)
                te1B = small.tile([128, 4], F32, tag="te1B")
                nc.vector.tensor_add(te1B, sv[:, :, 0], sv[:, :, 1])
                nc.vector.tensor_sub(te1B, te1B, e_iiB)
                s2v = sume2_all.rearrange("p (b t) -> p b t", t=2)
                te2B = small.tile([128, 4], F32, tag="te2B")
                nc.vector.tensor_add(te2B, s2v[:, :, 0], s2v[:, :, 1])
                eii2 = small.tile([128, 4], F32, tag="eii2")
                nc.vector.tensor_mul(eii2, e_iiB, e_iiB)
                nc.vector.tensor_sub(te2B, te2B, eii2)

                # S = tot - own, Q likewise; A = own (diag already zeroed)
                sB = small.tile([128, 4], F32, tag="sB")
                nc.vector.tensor_sub(sB, te1B, owne1B)
                rsB = small.tile([128, 4], F32, tag="rsB")
                nc.vector.reciprocal(rsB, sB)
                qB = small.tile([128, 4], F32, tag="qB")
                nc.vector.tensor_sub(qB, te2B, owne2B)
                qosB = small.tile([128, 4], F32, tag="qosB")
                nc.vector.tensor_mul(qosB, qB, rsB)
                pownB = small.tile([128, 4], F32, tag="pownB")
                for b in range(4):
                    nc.vector.tensor_copy(pownB[:, b:b + 1], store[f"pown{b}"])
                ctermB = small.tile([128, 4], F32, tag="ctermB")
                nc.vector.tensor_mul(ctermB, pownB, owne1B)
                nc.vector.tensor_sub(ctermB, cfullB, ctermB)
                denomB = small.tile([128, 4], F32, tag="denomB")
                nc.vector.tensor_add(denomB, qosB, ctermB)
                nc.vector.tensor_scalar_mul(denomB, denomB, ALPHA)
                nc.vector.tensor_add(denomB, denomB, owne1B)
                ldnB = small.tile([128, 4], F32, tag="ldnB")
                nc.scalar.activation(ldnB, denomB, AF.Ln)
                # mlpp = (ownz - zdg - 31*10)/31 - ln(denom)
                ptB = small.tile([128, 4], F32, tag="ptB")
                nc.vector.tensor_sub(ptB, ownzB, zdgB)
                nc.vector.tensor_scalar(
                    out=ptB, in0=ptB, scalar1=-31.0 * INV_T, scalar2=1.0 / 31.0,
                    op0=OP.add, op1=OP.mult)
                nc.vector.tensor_sub(ptB, ptB, ldnB)
                nc.vector.tensor_copy(stack[:, 0:4], ptB)
                # loss_pn partials: li = downp + ln(sumexp) - dmin (query blocks)
                lsum = small.tile([128, 2], F32, tag="lsum")
                nc.scalar.activation(lsum, sume_pn[:, 2:4], AF.Ln)
                liB = small.tile([128, 2], F32, tag="liB")
                nc.vector.tensor_add(liB, downpB[:, 2:4], lsum)
                nc.vector.tensor_sub(liB, liB, dminB[:, 2:4])
                nc.vector.tensor_copy(stack[:, 4:6], liB)

                ssum_ps = psF.tile([NSTAT, 1], F32, tag="ssum")
                nc.tensor.matmul(ssum_ps, lhsT=stack, rhs=ones_c,
                                 start=True, stop=True)
                ssum = small.tile([NSTAT, 1], F32, tag="ssum_sb")
                nc.vector.tensor_copy(ssum, ssum_ps)
                nc.sync.dma_start(out=stats_d[0:NSTAT], in_=ssum)

                dbg = work.tile([128, 16], F32, tag="dbgt")
                nc.vector.memset(dbg, 0.0)
                for i, t in enumerate([te1B, te2B, owne1B, owne2B, cfullB,
                                       denomB]):
                    nc.vector.tensor_copy(dbg[:, i:i + 1], t[:, 0:1])
                nc.sync.dma_start(out=dbg_d[:, :], in_=dbg)


# =========================================================
# Host side
# =========================================================
_NC_CACHE = None


def _build():
    global _NC_CACHE
    if _NC_CACHE is None:
        nc = bacc.Bacc(None, num_devices=NCORES)
        _emit(nc)
        nc.finalize()
        _NC_CACHE = nc
    return _NC_CACHE


def _consts_np():
    r = np.arange(128)
    c = np.arange(128)
    i128 = np.eye(128, dtype=np.float32)
    omi = (1.0 - i128).astype(np.float32)
    bd = (r[:, None] // 16 == c[None, :] // 16).astype(np.float32)
    oh = np.zeros((128, 128), np.float32)
    for par in range(2):
        own = 8 * par + r // 16
        for p in range(128):
            oh[p, 32 * par + own[p]] = 1.0
            oh[p, 32 * par + 16 + own[p]] = 1.0
    ownp = np.zeros((2, 128, 128), np.float32)
    for par in range(2):
        own = 8 * par + r // 16
        ownp[par] = (c[None, :] == own[:, None]).astype(np.float32)
    sel = np.zeros((128, 128), np.float32)
    for p in range(128):
        sel[p, p // 16] = 1.0
    cf = np.stack([i128, omi, bd, bd, oh, ownp[0], ownp[1], sel])
    cb = np.stack([bd, bd, omi])
    return cf, cb


def _rot_indices(k):
    idx = []
    for sp in range(NCORES):
        cc = (k + sp) % NCORES
        idx.extend(range(256 * cc, 256 * cc + 256))            # support
        idx.extend(range(2048 + 256 * cc, 2048 + 256 * cc + 256))  # query
    return np.asarray(idx, dtype=np.int64)


def _in_maps(tasks_em):
    import ml_dtypes
    cf, cb_f32 = _consts_np()
    cb = cb_f32.astype(ml_dtypes.bfloat16)
    in_maps = []
    for k in range(NCORES):
        em_rot = np.ascontiguousarray(tasks_em[:, _rot_indices(k), :],
                                      dtype=np.float32)
        in_maps.append({"em": em_rot, "cf": cf, "cb": cb})
    return in_maps


def _combine(stats):
    mlpp_sum = stats[:, 0:4].sum(dtype=np.float64)
    loss_pn = stats[:, 4:6].sum(dtype=np.float64) / 2048.0
    acc = stats[:, 6:8].sum(dtype=np.float64) / 2048.0
    pair_loss = (stats[0, 8] - 128.0) / 16256.0
    con_loss = -mlpp_sum / 4096.0
    loss = loss_pn + BETA * pair_loss + GAMMA * con_loss
    return (np.float32(loss), np.float32(acc))


def kernel(tasks_em, nway=128, kshot=16, qshot=16, repeat=2, **_kw):
    tasks_em = np.asarray(tasks_em, dtype=np.float32)
    assert tasks_em.shape == (2, 4096, 256)
    nc = _build()
    res = run_bass_kernel_spmd(nc, _in_maps(tasks_em), list(range(NCORES)))
    stats = np.stack([np.asarray(res.results[i]["stats"]) for i in range(NCORES)])
    return _combine(stats)


if __name__ == "__main__":
    nc = _build()
    print("built ok")
